# revision 14
# baseline (speedup 1.0000x reference)
"""Multi-head attention (B=2, S=2048, H=16, dh=64, D=1024) on 8 trn2 cores.

Sharding: core c -> (batch b = c // 4, head-group hg = c % 4).  Each core
computes 4 heads of one batch element: QKV projections for its head slice,
attention + softmax, attn_prob output, and a partial output projection.
Host sums the 4 per-batch partials (+bo) and concatenates attn_prob.
"""

import numpy as np
from contextlib import ExitStack

import concourse.bass as bass
import concourse.mybir as mybir
import concourse.tile as tile
from concourse import bacc
from concourse.bass_utils import run_bass_kernel_spmd
from concourse.masks import make_identity

f32 = mybir.dt.float32
f32r = mybir.dt.float32r
u8 = mybir.dt.uint8
bf16 = mybir.dt.bfloat16
AF = mybir.ActivationFunctionType

N_HEAD = 16
D_HEAD = 64
D_HIDN = 1024
B = 2
N_CORES = 8
NH = 4          # heads per core
HB = NH * D_HEAD  # 256: head-block width per core
FC = D_HIDN // 128  # 8 feature chunks
KC = HB // 128      # 2 chunks of the head block


def _r(ap):
    return ap.bitcast(f32r)


def build_program(S=2048, masked=False, num_devices=8, stages="ABC"):
    NT = S // 128          # token tiles
    PH = min(1024, S)      # psA tile width (ACT op width)
    NPH = S // PH          # psA tiles per full row
    C = min(512, PH)       # matmul N chunk (f32-out)
    NCH = PH // C          # matmul chunks per psA tile
    NC_S = S // C          # matmul chunks per full row
    PH2 = PH               # psA tile width (f32 psum)
    NPH2 = S // PH2
    CB = C                 # matmul N chunk (f32-out cap)

    nc = bacc.Bacc("TRN2", target_bir_lowering=False, debug=False,
                   num_devices=num_devices)
    Xq = nc.dram_tensor("Xq", [S, D_HIDN], f32, kind="ExternalInput").ap()
    Xk = nc.dram_tensor("Xk", [S, D_HIDN], f32, kind="ExternalInput").ap()
    Xv = nc.dram_tensor("Xv", [S, D_HIDN], f32, kind="ExternalInput").ap()
    Wq = nc.dram_tensor("Wq", [D_HIDN, HB], f32, kind="ExternalInput").ap()
    Wk = nc.dram_tensor("Wk", [D_HIDN, HB], f32, kind="ExternalInput").ap()
    Wv = nc.dram_tensor("Wv", [D_HIDN, HB], f32, kind="ExternalInput").ap()
    Wo = nc.dram_tensor("Wo", [HB, D_HIDN], f32, kind="ExternalInput").ap()
    bq = nc.dram_tensor("bq", [1, HB], f32, kind="ExternalInput").ap()
    bk = nc.dram_tensor("bk", [1, HB], f32, kind="ExternalInput").ap()
    bv = nc.dram_tensor("bv", [1, HB], f32, kind="ExternalInput").ap()
    attn = nc.dram_tensor("attn", [NH, S, S], f32, kind="ExternalOutput").ap()
    outp = nc.dram_tensor("outp", [S, D_HIDN], f32, kind="ExternalOutput").ap()
    if masked:
        mask = nc.dram_tensor("mask", [S, S], u8, kind="ExternalInput").ap()
        maskT = nc.dram_tensor("maskT", [S, S], u8, kind="ExternalInput").ap()

    with tile.TileContext(nc) as tc, ExitStack() as top:
        const = top.enter_context(tc.tile_pool(name="const", bufs=1))
        ident = const.tile([128, 128], f32, tag="ident", name="ident")
        make_identity(nc, ident[:])
        ones = const.tile([1, max(S, 512)], f32, tag="ones", name="ones")
        nc.gpsimd.memset(ones[:], 1.0)
        if masked:
            negcb = const.tile([128, PH2], f32, tag="negc", name="negcb")
            nc.gpsimd.memset(negcb[:], -1e9)

        pers = top.enter_context(tc.tile_pool(name="pers", bufs=1))
        qTb = [pers.tile([128, S], bf16, tag=f"qTb{i}", name=f"qTb{i}") for i in range(KC)]
        kTb = [pers.tile([128, S], bf16, tag=f"kTb{i}", name=f"kTb{i}") for i in range(KC)]
        ctx_sb = [pers.tile([128, S], bf16, tag=f"ctx{i}", name=f"ctx{i}") for i in range(KC)]
        vts = [pers.tile([128, NH * 65], bf16, tag=f"va{t}", name=f"va{t}") for t in range(NT)]
        recipZ = [pers.tile([128, NT], f32, tag=f"rz{h}", name=f"rz{h}") for h in range(NH)]
        negLnZ = [pers.tile([128, NT], f32, tag=f"nlz{h}", name=f"nlz{h}") for h in range(NH)]

        psA = top.enter_context(tc.tile_pool(name="psA", bufs=2, space="PSUM"))
        psB = top.enter_context(tc.tile_pool(name="psB", bufs=4, space="PSUM"))

        # ---------------- Stage A: transposes + projections ----------------
        with ExitStack() as sa:
            xin = sa.enter_context(tc.tile_pool(name="xin", bufs=6))
            xt = sa.enter_context(tc.tile_pool(name="xt", bufs=FC))
            wpool = sa.enter_context(tc.tile_pool(name="w", bufs=FC))
            bpool = sa.enter_context(tc.tile_pool(name="bias", bufs=3))

            def transpose_input(Xdram):
                # Load X in full-width token tiles (big DMAs), transpose in
                # groups of 4 token tiles so PSUM evacuations are [128, 512].
                XTc = [xt.tile([128, S], f32, tag="xt", name=f"xtf{f}")
                       for f in range(FC)]
                G = min(4, NT)
                for tg in range(NT // G):
                    xbs = []
                    for tt in range(G):
                        t = tg * G + tt
                        xb = xin.tile([128, D_HIDN], f32, tag="xin", name="xb")
                        nc.sync.dma_start(
                            out=xb[:], in_=Xdram[t * 128:(t + 1) * 128, :])
                        xbs.append(xb)
                    for f in range(FC):
                        ps = psA.tile([128, 128 * G], f32, tag="psA",
                                      name="psA_t")
                        for tt in range(G):
                            nc.tensor.matmul(
                                ps[:, tt * 128:(tt + 1) * 128],
                                lhsT=xbs[tt][:, f * 128:(f + 1) * 128],
                                rhs=ident[:],
                                is_transpose=True, start=True, stop=True,
                                skip_group_check=True)
                        nc.vector.tensor_copy(
                            out=_r(XTc[f][:, tg * 128 * G:(tg + 1) * 128 * G]),
                            in_=ps[:])
                return XTc

            def load_w(Wdram, bdram):
                wts = []
                for f in range(FC):
                    wt = wpool.tile([128, HB], f32, tag="w", name="wt")
                    nc.sync.dma_start(out=_r(wt[:]),
                                      in_=_r(Wdram[f * 128:(f + 1) * 128, :]))
                    wts.append(wt)
                bt = bpool.tile([1, HB], f32, tag="bias", name="bt")
                nc.sync.dma_start(out=_r(bt[:]), in_=_r(bdram[:]))
                return wts, bt

            def proj_qk(XTc, wts, bt, outtilesb):
                for m in range(KC):
                    for n in range(NC_S):
                        acc = psB.tile([128, C], f32, tag="psB", name="acc_qk")
                        for f in range(FC):
                            nc.tensor.matmul(
                                acc[:],
                                lhsT=_r(wts[f][:, m * 128:(m + 1) * 128]),
                                rhs=_r(XTc[f][:, n * C:(n + 1) * C]),
                                start=(f == 0), stop=False)
                        nc.tensor.matmul(
                            acc[:], lhsT=_r(bt[:, m * 128:(m + 1) * 128]),
                            rhs=_r(ones[:, n * C:(n + 1) * C]),
                            start=False, stop=True)
                        nc.vector.tensor_copy(
                            out=outtilesb[m][:, n * C:(n + 1) * C],
                            in_=acc[:])

            def proj_v(XTc, wts, bt):
                for t in range(NT):
                    acc = psB.tile([128, HB], f32, tag="psB", name="acc_v")
                    for f in range(FC):
                        nc.tensor.matmul(
                            acc[:],
                            lhsT=_r(XTc[f][:, t * 128:(t + 1) * 128]),
                            rhs=_r(wts[f][:]), start=(f == 0), stop=False)
                    nc.tensor.matmul(
                        acc[:], lhsT=_r(ones[:, t * 128:(t + 1) * 128]),
                        rhs=_r(bt[:]), start=False, stop=True)
                    va = vts[t]
                    for h in range(NH):
                        nc.vector.tensor_copy(
                            out=va[:, h * 65:h * 65 + 64],
                            in_=acc[:, h * 64:(h + 1) * 64])
                    vones = va[:].rearrange("p (h c) -> p h c", c=65)[:, :, 64:65]
                    nc.vector.memset(vones, 1.0)

            if "A" in stages:
                XTv = transpose_input(Xv)
                wv_t, bv_t = load_w(Wv, bv)
                proj_v(XTv, wv_t, bv_t)
                XTq = transpose_input(Xq)
                wq_t, bq_t = load_w(Wq, bq)
                proj_qk(XTq, wq_t, bq_t, qTb)
                XTk = transpose_input(Xk)
                wk_t, bk_t = load_w(Wk, bk)
                proj_qk(XTk, wk_t, bk_t, kTb)

        # ---------------- Stage B: attention ----------------
        with ExitStack() as sb:
            est = sb.enter_context(tc.tile_pool(name="est", bufs=3))
            pout = sb.enter_context(tc.tile_pool(name="pout", bufs=3))
            oout = sb.enter_context(tc.tile_pool(name="oout", bufs=5))
            zpool = sb.enter_context(tc.tile_pool(name="zrow", bufs=1))
            wopool = sb.enter_context(tc.tile_pool(name="wo", bufs=KC))
            if masked:
                mpool = sb.enter_context(tc.tile_pool(name="mload", bufs=2))

            def pass1(h):
                ti = h // 2
                hoff = (h % 2) * 64
                ctx_ps = [psB.tile([65, C], f32, tag="psB", name=f"ctx_ps{h}")
                          for _ in range(NC_S)]
                for kt in range(NT):
                    if masked:
                        mts = []
                        for ph in range(NPH2):
                            mt = mpool.tile([128, PH2], u8, tag="mload", name="mt")
                            nc.sync.dma_start(
                                out=mt[:],
                                in_=maskT[kt * 128:(kt + 1) * 128,
                                          ph * PH2:(ph + 1) * PH2])
                            mts.append(mt)
                    for ph in range(NPH2):
                        stp = psA.tile([128, PH2], f32, tag="psA",
                                       name="stp")
                        for cc in range(PH2 // CB):
                            nc.tensor.matmul(
                                stp[:, cc * CB:(cc + 1) * CB],
                                lhsT=kTb[ti][hoff:hoff + 64,
                                             kt * 128:(kt + 1) * 128],
                                rhs=qTb[ti][hoff:hoff + 64,
                                            ph * PH2 + cc * CB:
                                            ph * PH2 + (cc + 1) * CB],
                                start=True, stop=True)
                        if masked:
                            nc.vector.copy_predicated(
                                stp[:], mts[ph][:], negcb[:])
                        eh = est.tile([128, PH2], bf16, tag="est", name="eh")
                        nc.scalar.activation(eh[:], stp[:], AF.Exp,
                                             scale=0.125)
                        for cc in range(PH2 // C):
                            nc.tensor.matmul(
                                ctx_ps[ph * (PH2 // C) + cc][:],
                                lhsT=vts[kt][:, h * 65:(h + 1) * 65],
                                rhs=eh[:, cc * C:(cc + 1) * C],
                                start=(kt == 0), stop=(kt == NT - 1))
                # evacuate ctx rows + Z row
                zrow = zpool.tile([1, S], f32, tag="zrow", name="zrow")
                for n in range(NC_S):
                    nc.vector.tensor_copy(
                        out=ctx_sb[ti][hoff:hoff + 64, n * C:(n + 1) * C],
                        in_=ctx_ps[n][0:64, :])
                    nc.vector.tensor_copy(
                        out=zrow[:, n * C:(n + 1) * C],
                        in_=ctx_ps[n][64:65, :])
                # transpose Z -> [128, NT]; recipZ, -lnZ
                zps = psB.tile([128, NT], f32, tag="psB", name="zps")
                for t in range(NT):
                    nc.tensor.matmul(
                        zps[:, t:t + 1],
                        lhsT=zrow[:, t * 128:(t + 1) * 128],
                        rhs=ones[0:1, 0:1], is_transpose=True,
                        start=True, stop=True,
                        skip_group_check=True)
                zt = oout.tile([128, NT], f32, tag="zt", name="zt")
                nc.vector.tensor_copy(out=zt[:], in_=zps[:])
                nc.vector.reciprocal(out=recipZ[h][:], in_=zt[:])
                nc.scalar.activation(negLnZ[h][:], recipZ[h][:], AF.Ln)

            def pass2(h):
                ti = h // 2
                hoff = (h % 2) * 64
                for qt in range(NT):
                    pt = pout.tile([128, S], f32, tag="pout", name="pt")
                    if masked:
                        mts = []
                        for ph in range(NPH2):
                            mt = mpool.tile([128, PH2], u8, tag="mload", name="mt")
                            nc.sync.dma_start(
                                out=mt[:],
                                in_=mask[qt * 128:(qt + 1) * 128,
                                         ph * PH2:(ph + 1) * PH2])
                            mts.append(mt)
                    for ph in range(NPH2):
                        sp = psA.tile([128, PH2], f32, tag="psA", name="sp")
                        for cc in range(PH2 // CB):
                            nc.tensor.matmul(
                                sp[:, cc * CB:(cc + 1) * CB],
                                lhsT=qTb[ti][hoff:hoff + 64,
                                             qt * 128:(qt + 1) * 128],
                                rhs=kTb[ti][hoff:hoff + 64,
                                            ph * PH2 + cc * CB:
                                            ph * PH2 + (cc + 1) * CB],
                                start=True, stop=True)
                        if masked:
                            nc.vector.copy_predicated(
                                sp[:], mts[ph][:], negcb[:])
                        nc.scalar.activation(
                            pt[:, ph * PH2:(ph + 1) * PH2], sp[:], AF.Exp,
                            scale=0.125, bias=negLnZ[h][:, qt:qt + 1])
                    nc.sync.dma_start(
                        out=attn[h, qt * 128:(qt + 1) * 128, :], in_=pt[:])

            for h in range(NH):
                if "1" in stages or "B" in stages:
                    pass1(h)
                if "2" in stages or "B" in stages:
                    pass2(h)

            # ---------------- Stage C: output projection ----------------
            if "C" not in stages:
                wo_ts = None
            wo_ts = []
            for kc in range(KC if "C" in stages else 0):
                wt = wopool.tile([128, D_HIDN], bf16, tag="wo", name="wo_t")
                nc.gpsimd.dma_start(out=wt[:],
                                    in_=Wo[kc * 128:(kc + 1) * 128, :])
                wo_ts.append(wt)
            NOC = D_HIDN // 512
            for qt in range(NT if "C" in stages else 0):
                obs = []
                for h in range(NH):
                    ti = h // 2
                    hoff = (h % 2) * 64
                    ob = oout.tile([128, D_HIDN], f32, tag="ob",
                                   name=f"ob{h}")
                    for n in range(NOC):
                        acc = psB.tile([128, 512], f32, tag="psB", name="acc_o")
                        nc.tensor.matmul(
                            acc[:],
                            lhsT=ctx_sb[ti][hoff:hoff + 64,
                                            qt * 128:(qt + 1) * 128],
                            rhs=wo_ts[ti][hoff:hoff + 64,
                                          n * 512:(n + 1) * 512],
                            start=True, stop=True)
                        nc.scalar.mul(
                            out=ob[:, n * 512:(n + 1) * 512], in_=acc[:],
                            mul=recipZ[h][:, qt:qt + 1])
                    obs.append(ob)
                nc.gpsimd.tensor_add(obs[0][:], obs[0][:], obs[1][:])
                nc.vector.tensor_add(obs[2][:], obs[2][:], obs[3][:])
                nc.vector.tensor_add(obs[0][:], obs[0][:], obs[2][:])
                nc.sync.dma_start(out=outp[qt * 128:(qt + 1) * 128, :],
                                  in_=obs[0][:])

    nc.finalize()
    return nc


_PROGRAM_CACHE = {}


def _get_program(S, masked):
    key = (S, masked)
    if key not in _PROGRAM_CACHE:
        _PROGRAM_CACHE[key] = build_program(S=S, masked=masked,
                                            num_devices=N_CORES)
    return _PROGRAM_CACHE[key]


def kernel(Q, K, V, attn_mask, Wq, bq, Wk, bk, Wv, bv, Wo, bo):
    Q = np.asarray(Q, np.float32)
    K = np.asarray(K, np.float32)
    V = np.asarray(V, np.float32)
    Wq = np.asarray(Wq, np.float32)
    Wk = np.asarray(Wk, np.float32)
    Wv = np.asarray(Wv, np.float32)
    Wo = np.asarray(Wo, np.float32)
    bq = np.asarray(bq, np.float32)
    bk = np.asarray(bk, np.float32)
    bv = np.asarray(bv, np.float32)
    bo = np.asarray(bo, np.float32)
    mask_np = np.asarray(attn_mask)
    b_, S, _ = Q.shape
    masked = bool(mask_np.any())
    nc = _get_program(S, masked)

    in_maps = []
    for c in range(N_CORES):
        b = c // 4
        hg = c % 4
        sl = slice(hg * HB, (hg + 1) * HB)
        m = {
            "Xq": np.ascontiguousarray(Q[b]),
            "Xk": np.ascontiguousarray(K[b]),
            "Xv": np.ascontiguousarray(V[b]),
            "Wq": np.ascontiguousarray(Wq[:, sl]),
            "Wk": np.ascontiguousarray(Wk[:, sl]),
            "Wv": np.ascontiguousarray(Wv[:, sl]),
            "Wo": np.ascontiguousarray(Wo[sl, :]),
            "bq": np.ascontiguousarray(bq[sl]).reshape(1, HB),
            "bk": np.ascontiguousarray(bk[sl]).reshape(1, HB),
            "bv": np.ascontiguousarray(bv[sl]).reshape(1, HB),
        }
        if masked:
            mu8 = mask_np[b].astype(np.uint8)
            m["mask"] = np.ascontiguousarray(mu8)
            m["maskT"] = np.ascontiguousarray(mu8.T)
        in_maps.append(m)

    res = run_bass_kernel_spmd(nc, in_maps, list(range(N_CORES)))

    output = np.zeros((b_, S, D_HIDN), np.float32)
    attn_prob = np.empty((b_, N_HEAD, S, S), np.float32)
    for c in range(N_CORES):
        b = c // 4
        hg = c % 4
        output[b] += res.results[c]["outp"]
        attn_prob[b, hg * NH:(hg + 1) * NH] = res.results[c]["attn"]
    output += bo[None, None, :]
    return output, attn_prob


# revision 15
# speedup vs baseline: 1.1070x; 1.1070x over previous
"""Multi-head attention (B=2, S=2048, H=16, dh=64, D=1024) on 8 trn2 cores.

Sharding: core c -> (batch b = c // 4, head-group hg = c % 4).  Each core
computes 4 heads of one batch element: QKV projections for its head slice,
attention + softmax, attn_prob output, and a partial output projection.
Host sums the 4 per-batch partials (+bo) and concatenates attn_prob.
"""

import numpy as np
from contextlib import ExitStack

import concourse.bass as bass
import concourse.mybir as mybir
import concourse.tile as tile
from concourse import bacc
from concourse.bass_utils import run_bass_kernel_spmd
from concourse.masks import make_identity

f32 = mybir.dt.float32
f32r = mybir.dt.float32r
u8 = mybir.dt.uint8
bf16 = mybir.dt.bfloat16
AF = mybir.ActivationFunctionType

N_HEAD = 16
D_HEAD = 64
D_HIDN = 1024
B = 2
N_CORES = 8
NH = 4          # heads per core
HB = NH * D_HEAD  # 256: head-block width per core
FC = D_HIDN // 128  # 8 feature chunks
KC = HB // 128      # 2 chunks of the head block


def _r(ap):
    return ap.bitcast(f32r)


def build_program(S=2048, masked=False, num_devices=8, stages="ABC"):
    NT = S // 128          # token tiles
    PH = min(1024, S)      # psA tile width (ACT op width)
    NPH = S // PH          # psA tiles per full row
    C = min(512, PH)       # matmul N chunk (f32-out)
    NCH = PH // C          # matmul chunks per psA tile
    NC_S = S // C          # matmul chunks per full row
    PH2 = PH               # psA tile width (f32 psum)
    NPH2 = S // PH2
    CB = C                 # matmul N chunk (f32-out cap)

    nc = bacc.Bacc("TRN2", target_bir_lowering=False, debug=False,
                   num_devices=num_devices)
    Xq = nc.dram_tensor("Xq", [S, D_HIDN], f32, kind="ExternalInput").ap()
    Xk = nc.dram_tensor("Xk", [S, D_HIDN], f32, kind="ExternalInput").ap()
    Xv = nc.dram_tensor("Xv", [S, D_HIDN], f32, kind="ExternalInput").ap()
    Wq = nc.dram_tensor("Wq", [D_HIDN, HB], f32, kind="ExternalInput").ap()
    Wk = nc.dram_tensor("Wk", [D_HIDN, HB], f32, kind="ExternalInput").ap()
    Wv = nc.dram_tensor("Wv", [D_HIDN, HB], f32, kind="ExternalInput").ap()
    Wo = nc.dram_tensor("Wo", [HB, D_HIDN], f32, kind="ExternalInput").ap()
    bq = nc.dram_tensor("bq", [1, HB], f32, kind="ExternalInput").ap()
    bk = nc.dram_tensor("bk", [1, HB], f32, kind="ExternalInput").ap()
    bv = nc.dram_tensor("bv", [1, HB], f32, kind="ExternalInput").ap()
    attn = nc.dram_tensor("attn", [NH, S, S], f32, kind="ExternalOutput").ap()
    outp = nc.dram_tensor("outp", [S, D_HIDN], f32, kind="ExternalOutput").ap()
    if masked:
        mask = nc.dram_tensor("mask", [S, S], u8, kind="ExternalInput").ap()
        maskT = nc.dram_tensor("maskT", [S, S], u8, kind="ExternalInput").ap()

    with tile.TileContext(nc) as tc, ExitStack() as top:
        const = top.enter_context(tc.tile_pool(name="const", bufs=1))
        ident = const.tile([128, 128], f32, tag="ident", name="ident")
        make_identity(nc, ident[:])
        ones = const.tile([1, max(S, 512)], f32, tag="ones", name="ones")
        nc.gpsimd.memset(ones[:], 1.0)
        if masked:
            negcb = const.tile([128, PH2], f32, tag="negc", name="negcb")
            nc.gpsimd.memset(negcb[:], -1e9)

        pers = top.enter_context(tc.tile_pool(name="pers", bufs=1))
        qTb = [pers.tile([128, S], bf16, tag=f"qTb{i}", name=f"qTb{i}") for i in range(KC)]
        kTb = [pers.tile([128, S], bf16, tag=f"kTb{i}", name=f"kTb{i}") for i in range(KC)]
        kTz = [pers.tile([128, S], bf16, tag=f"kTz{h}", name=f"kTz{h}") for h in range(NH)]
        qTz = [pers.tile([128, S], bf16, tag=f"qTz{h}", name=f"qTz{h}") for h in range(NH)]
        ctx_sb = [pers.tile([128, S], bf16, tag=f"ctx{i}", name=f"ctx{i}") for i in range(KC)]
        vts = [pers.tile([128, NH * 128], bf16, tag=f"va{t}", name=f"va{t}") for t in range(NT)]
        recipZ = [pers.tile([128, NT], f32, tag=f"rz{h}", name=f"rz{h}") for h in range(NH)]
        negLnZ = [pers.tile([128, NT], f32, tag=f"nlz{h}", name=f"nlz{h}") for h in range(NH)]

        psA = top.enter_context(tc.tile_pool(name="psA", bufs=2, space="PSUM"))
        psB = top.enter_context(tc.tile_pool(name="psB", bufs=4, space="PSUM"))

        # ---------------- Stage A: transposes + projections ----------------
        with ExitStack() as sa:
            xin = sa.enter_context(tc.tile_pool(name="xin", bufs=6))
            xt = sa.enter_context(tc.tile_pool(name="xt", bufs=FC))
            wpool = sa.enter_context(tc.tile_pool(name="w", bufs=FC))
            bpool = sa.enter_context(tc.tile_pool(name="bias", bufs=3))

            def transpose_input(Xdram):
                # Load X in full-width token tiles (big DMAs), transpose in
                # groups of 4 token tiles so PSUM evacuations are [128, 512].
                XTc = [xt.tile([128, S], f32, tag="xt", name=f"xtf{f}")
                       for f in range(FC)]
                G = min(4, NT)
                for tg in range(NT // G):
                    xbs = []
                    for tt in range(G):
                        t = tg * G + tt
                        xb = xin.tile([128, D_HIDN], f32, tag="xin", name="xb")
                        nc.sync.dma_start(
                            out=xb[:], in_=Xdram[t * 128:(t + 1) * 128, :])
                        xbs.append(xb)
                    for f in range(FC):
                        ps = psA.tile([128, 128 * G], f32, tag="psA",
                                      name="psA_t")
                        for tt in range(G):
                            nc.tensor.matmul(
                                ps[:, tt * 128:(tt + 1) * 128],
                                lhsT=xbs[tt][:, f * 128:(f + 1) * 128],
                                rhs=ident[:],
                                is_transpose=True, start=True, stop=True,
                                skip_group_check=True)
                        nc.vector.tensor_copy(
                            out=_r(XTc[f][:, tg * 128 * G:(tg + 1) * 128 * G]),
                            in_=ps[:])
                return XTc

            def load_w(Wdram, bdram):
                wts = []
                for f in range(FC):
                    wt = wpool.tile([128, HB], f32, tag="w", name="wt")
                    nc.sync.dma_start(out=_r(wt[:]),
                                      in_=_r(Wdram[f * 128:(f + 1) * 128, :]))
                    wts.append(wt)
                bt = bpool.tile([1, HB], f32, tag="bias", name="bt")
                nc.sync.dma_start(out=_r(bt[:]), in_=_r(bdram[:]))
                return wts, bt

            def proj_qk(XTc, wts, bt, outtilesb):
                for m in range(KC):
                    for n in range(NC_S):
                        acc = psB.tile([128, C], f32, tag="psB", name="acc_qk")
                        for f in range(FC):
                            nc.tensor.matmul(
                                acc[:],
                                lhsT=_r(wts[f][:, m * 128:(m + 1) * 128]),
                                rhs=_r(XTc[f][:, n * C:(n + 1) * C]),
                                start=(f == 0), stop=False)
                        nc.tensor.matmul(
                            acc[:], lhsT=_r(bt[:, m * 128:(m + 1) * 128]),
                            rhs=_r(ones[:, n * C:(n + 1) * C]),
                            start=False, stop=True)
                        nc.vector.tensor_copy(
                            out=outtilesb[m][:, n * C:(n + 1) * C],
                            in_=acc[:])

            def proj_v(XTc, wts, bt):
                for t in range(NT):
                    acc = psB.tile([128, HB], f32, tag="psB", name="acc_v")
                    for f in range(FC):
                        nc.tensor.matmul(
                            acc[:],
                            lhsT=_r(XTc[f][:, t * 128:(t + 1) * 128]),
                            rhs=_r(wts[f][:]), start=(f == 0), stop=False)
                    nc.tensor.matmul(
                        acc[:], lhsT=_r(ones[:, t * 128:(t + 1) * 128]),
                        rhs=_r(bt[:]), start=False, stop=True)
                    va = vts[t]
                    nc.vector.memset(va[:, :], 0.0)
                    for h in range(NH):
                        nc.vector.tensor_copy(
                            out=va[:, h * 128:h * 128 + 64],
                            in_=acc[:, h * 64:(h + 1) * 64])
                    vones = va[:].rearrange("p (h c) -> p h c", c=128)[:, :, 64:65]
                    nc.vector.memset(vones, 1.0)

            if "A" in stages:
                XTv = transpose_input(Xv)
                wv_t, bv_t = load_w(Wv, bv)
                proj_v(XTv, wv_t, bv_t)
                XTq = transpose_input(Xq)
                wq_t, bq_t = load_w(Wq, bq)
                proj_qk(XTq, wq_t, bq_t, qTb)
                for h in range(NH):
                    ti = h // 2
                    hoff = (h % 2) * 64
                    nc.vector.memset(qTz[h][64:128, :], 0.0)
                    nc.vector.tensor_copy(
                        out=qTz[h][0:64, :],
                        in_=qTb[ti][hoff:hoff + 64, :])
                XTk = transpose_input(Xk)
                wk_t, bk_t = load_w(Wk, bk)
                proj_qk(XTk, wk_t, bk_t, kTb)
                for h in range(NH):
                    ti = h // 2
                    hoff = (h % 2) * 64
                    nc.vector.memset(kTz[h][64:128, :], 0.0)
                    nc.vector.tensor_copy(
                        out=kTz[h][0:64, :],
                        in_=kTb[ti][hoff:hoff + 64, :])

        # ---------------- Stage B: attention ----------------
        with ExitStack() as sb:
            est = sb.enter_context(tc.tile_pool(name="est", bufs=3))
            pout = sb.enter_context(tc.tile_pool(name="pout", bufs=3))
            oout = sb.enter_context(tc.tile_pool(name="oout", bufs=5))
            zpool = sb.enter_context(tc.tile_pool(name="zrow", bufs=1))
            wopool = sb.enter_context(tc.tile_pool(name="wo", bufs=KC))
            if masked:
                mpool = sb.enter_context(tc.tile_pool(name="mload", bufs=2))

            def pass1(h):
                ti = h // 2
                hoff = (h % 2) * 64
                ctx_ps = [psB.tile([128, C], f32, tag="psB", name=f"ctx_ps{h}")
                          for _ in range(NC_S)]
                for kt in range(NT):
                    if masked:
                        mts = []
                        for ph in range(NPH2):
                            mt = mpool.tile([128, PH2], u8, tag="mload", name="mt")
                            nc.sync.dma_start(
                                out=mt[:],
                                in_=maskT[kt * 128:(kt + 1) * 128,
                                          ph * PH2:(ph + 1) * PH2])
                            mts.append(mt)
                    for ph in range(NPH2):
                        stp = psA.tile([128, PH2], f32, tag="psA",
                                       name="stp")
                        for cc in range(PH2 // CB):
                            nc.tensor.matmul(
                                stp[:, cc * CB:(cc + 1) * CB],
                                lhsT=kTz[h][:, kt * 128:(kt + 1) * 128],
                                rhs=qTz[h][:, ph * PH2 + cc * CB:
                                           ph * PH2 + (cc + 1) * CB],
                                start=True, stop=True)
                        if masked:
                            nc.vector.copy_predicated(
                                stp[:], mts[ph][:], negcb[:])
                        eh = est.tile([128, PH2], bf16, tag="est", name="eh")
                        nc.scalar.activation(eh[:], stp[:], AF.Exp,
                                             scale=0.125)
                        for cc in range(PH2 // C):
                            nc.tensor.matmul(
                                ctx_ps[ph * (PH2 // C) + cc][:],
                                lhsT=vts[kt][:, h * 128:(h + 1) * 128],
                                rhs=eh[:, cc * C:(cc + 1) * C],
                                start=(kt == 0), stop=(kt == NT - 1))
                # evacuate ctx rows + Z row
                zrow = zpool.tile([1, S], f32, tag="zrow", name="zrow")
                for n in range(NC_S):
                    nc.vector.tensor_copy(
                        out=ctx_sb[ti][hoff:hoff + 64, n * C:(n + 1) * C],
                        in_=ctx_ps[n][0:64, :])
                    nc.vector.tensor_copy(
                        out=zrow[:, n * C:(n + 1) * C],
                        in_=ctx_ps[n][64:65, :])
                # transpose Z -> [128, NT]; recipZ, -lnZ
                zps = psB.tile([128, NT], f32, tag="psB", name="zps")
                for t in range(NT):
                    nc.tensor.matmul(
                        zps[:, t:t + 1],
                        lhsT=zrow[:, t * 128:(t + 1) * 128],
                        rhs=ones[0:1, 0:1], is_transpose=True,
                        start=True, stop=True,
                        skip_group_check=True)
                zt = oout.tile([128, NT], f32, tag="zt", name="zt")
                nc.vector.tensor_copy(out=zt[:], in_=zps[:])
                nc.vector.reciprocal(out=recipZ[h][:], in_=zt[:])
                nc.scalar.activation(negLnZ[h][:], recipZ[h][:], AF.Ln)

            def pass2(h):
                ti = h // 2
                hoff = (h % 2) * 64
                for qt in range(NT):
                    pt = pout.tile([128, S], f32, tag="pout", name="pt")
                    if masked:
                        mts = []
                        for ph in range(NPH2):
                            mt = mpool.tile([128, PH2], u8, tag="mload", name="mt")
                            nc.sync.dma_start(
                                out=mt[:],
                                in_=mask[qt * 128:(qt + 1) * 128,
                                         ph * PH2:(ph + 1) * PH2])
                            mts.append(mt)
                    for ph in range(NPH2):
                        sp = psA.tile([128, PH2], f32, tag="psA", name="sp")
                        for cc in range(PH2 // CB):
                            nc.tensor.matmul(
                                sp[:, cc * CB:(cc + 1) * CB],
                                lhsT=qTz[h][:, qt * 128:(qt + 1) * 128],
                                rhs=kTz[h][:, ph * PH2 + cc * CB:
                                           ph * PH2 + (cc + 1) * CB],
                                start=True, stop=True)
                        if masked:
                            nc.vector.copy_predicated(
                                sp[:], mts[ph][:], negcb[:])
                        nc.scalar.activation(
                            pt[:, ph * PH2:(ph + 1) * PH2], sp[:], AF.Exp,
                            scale=0.125, bias=negLnZ[h][:, qt:qt + 1])
                    nc.sync.dma_start(
                        out=attn[h, qt * 128:(qt + 1) * 128, :], in_=pt[:])

            for h in range(NH):
                if "1" in stages or "B" in stages:
                    pass1(h)
                if "2" in stages or "B" in stages:
                    pass2(h)

            # ---------------- Stage C: output projection ----------------
            if "C" not in stages:
                wo_ts = None
            wo_ts = []
            for kc in range(KC if "C" in stages else 0):
                wt = wopool.tile([128, D_HIDN], bf16, tag="wo", name="wo_t")
                nc.gpsimd.dma_start(out=wt[:],
                                    in_=Wo[kc * 128:(kc + 1) * 128, :])
                wo_ts.append(wt)
            NOC = D_HIDN // 512
            for qt in range(NT if "C" in stages else 0):
                obs = []
                for h in range(NH):
                    ti = h // 2
                    hoff = (h % 2) * 64
                    ob = oout.tile([128, D_HIDN], f32, tag="ob",
                                   name=f"ob{h}")
                    for n in range(NOC):
                        acc = psB.tile([128, 512], f32, tag="psB", name="acc_o")
                        nc.tensor.matmul(
                            acc[:],
                            lhsT=ctx_sb[ti][hoff:hoff + 64,
                                            qt * 128:(qt + 1) * 128],
                            rhs=wo_ts[ti][hoff:hoff + 64,
                                          n * 512:(n + 1) * 512],
                            start=True, stop=True)
                        nc.scalar.mul(
                            out=ob[:, n * 512:(n + 1) * 512], in_=acc[:],
                            mul=recipZ[h][:, qt:qt + 1])
                    obs.append(ob)
                nc.gpsimd.tensor_add(obs[0][:], obs[0][:], obs[1][:])
                nc.vector.tensor_add(obs[2][:], obs[2][:], obs[3][:])
                nc.vector.tensor_add(obs[0][:], obs[0][:], obs[2][:])
                nc.sync.dma_start(out=outp[qt * 128:(qt + 1) * 128, :],
                                  in_=obs[0][:])

    nc.finalize()
    return nc


_PROGRAM_CACHE = {}


def _get_program(S, masked):
    key = (S, masked)
    if key not in _PROGRAM_CACHE:
        _PROGRAM_CACHE[key] = build_program(S=S, masked=masked,
                                            num_devices=N_CORES)
    return _PROGRAM_CACHE[key]


def kernel(Q, K, V, attn_mask, Wq, bq, Wk, bk, Wv, bv, Wo, bo):
    Q = np.asarray(Q, np.float32)
    K = np.asarray(K, np.float32)
    V = np.asarray(V, np.float32)
    Wq = np.asarray(Wq, np.float32)
    Wk = np.asarray(Wk, np.float32)
    Wv = np.asarray(Wv, np.float32)
    Wo = np.asarray(Wo, np.float32)
    bq = np.asarray(bq, np.float32)
    bk = np.asarray(bk, np.float32)
    bv = np.asarray(bv, np.float32)
    bo = np.asarray(bo, np.float32)
    mask_np = np.asarray(attn_mask)
    b_, S, _ = Q.shape
    masked = bool(mask_np.any())
    nc = _get_program(S, masked)

    in_maps = []
    for c in range(N_CORES):
        b = c // 4
        hg = c % 4
        sl = slice(hg * HB, (hg + 1) * HB)
        m = {
            "Xq": np.ascontiguousarray(Q[b]),
            "Xk": np.ascontiguousarray(K[b]),
            "Xv": np.ascontiguousarray(V[b]),
            "Wq": np.ascontiguousarray(Wq[:, sl]),
            "Wk": np.ascontiguousarray(Wk[:, sl]),
            "Wv": np.ascontiguousarray(Wv[:, sl]),
            "Wo": np.ascontiguousarray(Wo[sl, :]),
            "bq": np.ascontiguousarray(bq[sl]).reshape(1, HB),
            "bk": np.ascontiguousarray(bk[sl]).reshape(1, HB),
            "bv": np.ascontiguousarray(bv[sl]).reshape(1, HB),
        }
        if masked:
            mu8 = mask_np[b].astype(np.uint8)
            m["mask"] = np.ascontiguousarray(mu8)
            m["maskT"] = np.ascontiguousarray(mu8.T)
        in_maps.append(m)

    res = run_bass_kernel_spmd(nc, in_maps, list(range(N_CORES)))

    output = np.zeros((b_, S, D_HIDN), np.float32)
    attn_prob = np.empty((b_, N_HEAD, S, S), np.float32)
    for c in range(N_CORES):
        b = c // 4
        hg = c % 4
        output[b] += res.results[c]["outp"]
        attn_prob[b, hg * NH:(hg + 1) * NH] = res.results[c]["attn"]
    output += bo[None, None, :]
    return output, attn_prob
